# revision 1
# baseline (speedup 1.0000x reference)
"""Trainium2 kernel for the FEM kinematic (strain) layer.

Reference computation:
    disp = inputs[:, elem_nodes]                      # [B, E, 8, 2]
    dd   = einsum('egkl,bekn->begnl', shpdx, disp)    # [B, E, 9, 2, 2]
    out  = stack([dd[...,0,0], dd[...,1,1],
                  0.5*(dd[...,0,1] + dd[...,1,0])])   # [B, E*9, 3]

Sharding: elements split across 8 NeuronCores.  The host resolves the
element->node indirection (index marshalling) and ships each core an
element-major displacement block; the device streams shpdx + disp and
computes the strain products with DVE, using the identity
    S1*u + S0*v = (S0+S1)*(u+v) - S0*u - S1*v
so only 3 elementwise products are needed per (element, gauss point).
"""

import sys
import numpy as np

sys.path.insert(0, "/opt/trn_rl_repo")

import concourse.bass as bass
import concourse.bacc as bacc
import concourse.mybir as mybir
import concourse.tile as tile
from concourse.bass_utils import run_bass_kernel_spmd

B = 4
N_NODES = 1_000_000
N_ELEM = 500_000
N_GP = 9
N_EN = 8
N_CORES = 8

E_CORE = N_ELEM // N_CORES            # 62500 elements per core
P = 128                               # SBUF partitions
C = 16                                # elements per partition per chunk
CHUNK = P * C                         # 2048 elements per chunk
N_CHUNKS = -(-E_CORE // CHUNK)        # 31
E_PAD = N_CHUNKS * CHUNK              # 63488 (988 pad elements)

_compiled = None


def _build_program():
    nc = bacc.Bacc("TRN2", target_bir_lowering=False, debug=False)
    f32 = mybir.dt.float32

    # [E_PAD, 144] f32: per element (g, k, l) row-major
    s_d = nc.dram_tensor("shp", [E_PAD, 144], f32, kind="ExternalInput").ap()
    # [E_PAD, 64] f32: per element (k, b, n) row-major
    d_d = nc.dram_tensor("disp", [E_PAD, 64], f32, kind="ExternalInput").ap()
    # [B, E_PAD*9, 3] f32
    o_d = nc.dram_tensor("out", [B, E_PAD * 9, 3], f32, kind="ExternalOutput").ap()

    s_v = s_d.rearrange("(n p c) f -> n p (c f)", p=P, c=C)
    d_v = d_d.rearrange("(n p c) f -> n p (c f)", p=P, c=C)
    # out view per (b, chunk): [P, C*27]
    o_v = o_d.rearrange("b (n p x) three -> b n p (x three)", p=P, x=C * 9)

    with tile.TileContext(nc) as tc:
        with (
            tc.tile_pool(name="io", bufs=3) as io_pool,
            tc.tile_pool(name="tmp", bufs=2) as tmp_pool,
        ):
            for i in range(N_CHUNKS):
                S = io_pool.tile([P, C * 144], f32, tag="S")
                D = io_pool.tile([P, C * 64], f32, tag="D")
                nc.sync.dma_start(out=S[:], in_=s_v[i])
                nc.sync.dma_start(out=D[:], in_=d_v[i])

                Sr = S[:].rearrange("p (c g k l) -> p c g k l", c=C, g=9, k=8, l=2)
                Dr = D[:].rearrange("p (c k b n) -> p c k b n", c=C, k=8, b=B, n=2)

                # A = S0 + S1, contiguous [p, (c g k)]
                A = tmp_pool.tile([P, C * 72], f32, tag="A")
                Av = A[:].rearrange("p (c g k) -> p c g k", c=C, g=9)
                nc.vector.tensor_tensor(
                    out=Av, in0=Sr[:, :, :, :, 0], in1=Sr[:, :, :, :, 1],
                    op=mybir.AluOpType.add,
                )

                O = io_pool.tile([P, B * C * 27], f32, tag="O")
                Ov = O[:].rearrange("p (b c g t) -> p b c g t", b=B, c=C, g=9, t=3)

                for b in range(B):
                    u = Dr[:, :, :, b, 0]          # [p, C, 8]
                    v = Dr[:, :, :, b, 1]

                    W = tmp_pool.tile([P, C * 8], f32, tag="W")
                    Wv = W[:].rearrange("p (c k) -> p c k", c=C)
                    nc.vector.tensor_tensor(
                        out=Wv, in0=u, in1=v, op=mybir.AluOpType.add
                    )

                    # broadcast displacement over g: [p, C, 1, 8] -> [p, C, 9, 8]
                    ub = u[:, :, None, :].to_broadcast([P, C, 9, 8])
                    vb = v[:, :, None, :].to_broadcast([P, C, 9, 8])
                    wb = Wv[:, :, None, :].to_broadcast([P, C, 9, 8])

                    T0 = tmp_pool.tile([P, C * 72], f32, tag="T")
                    T1 = tmp_pool.tile([P, C * 72], f32, tag="T")
                    T2 = tmp_pool.tile([P, C * 72], f32, tag="T")
                    T0v = T0[:].rearrange("p (c g k) -> p c g k", c=C, g=9)
                    T1v = T1[:].rearrange("p (c g k) -> p c g k", c=C, g=9)
                    T2v = T2[:].rearrange("p (c g k) -> p c g k", c=C, g=9)

                    nc.vector.tensor_tensor(
                        out=T0v, in0=Sr[:, :, :, :, 0], in1=ub,
                        op=mybir.AluOpType.mult,
                    )
                    nc.vector.tensor_tensor(
                        out=T1v, in0=Sr[:, :, :, :, 1], in1=vb,
                        op=mybir.AluOpType.mult,
                    )
                    nc.vector.tensor_tensor(
                        out=T2v, in0=Av, in1=wb, op=mybir.AluOpType.mult,
                    )

                    # xx / yy land directly in the (strided) output staging
                    nc.vector.reduce_sum(
                        out=Ov[:, b, :, :, 0], in_=T0v, axis=mybir.AxisListType.X
                    )
                    nc.vector.reduce_sum(
                        out=Ov[:, b, :, :, 1], in_=T1v, axis=mybir.AxisListType.X
                    )

                    R = tmp_pool.tile([P, C * 9], f32, tag="R")
                    Rv = R[:].rearrange("p (c g) -> p c g", c=C)
                    nc.vector.reduce_sum(out=Rv, in_=T2v, axis=mybir.AxisListType.X)
                    nc.vector.tensor_tensor(
                        out=Rv, in0=Rv, in1=Ov[:, b, :, :, 0],
                        op=mybir.AluOpType.subtract,
                    )
                    nc.vector.tensor_tensor(
                        out=Rv, in0=Rv, in1=Ov[:, b, :, :, 1],
                        op=mybir.AluOpType.subtract,
                    )
                    nc.scalar.activation(
                        out=Ov[:, b, :, :, 2], in_=Rv,
                        func=mybir.ActivationFunctionType.Copy, scale=0.5,
                    )

                for b in range(B):
                    nc.sync.dma_start(
                        out=o_v[b, i],
                        in_=O[:, b * C * 27:(b + 1) * C * 27],
                    )

    nc.compile()
    return nc


def _get_program():
    global _compiled
    if _compiled is None:
        _compiled = _build_program()
    return _compiled


def kernel(inputs, shpdx, elem_nodes, _want_trace=False):
    nc = _get_program()

    # Host-side index marshalling: resolve element->node indirection and
    # build per-core element-major blocks.
    in_maps = []
    for c in range(N_CORES):
        sl = slice(c * E_CORE, (c + 1) * E_CORE)
        en = elem_nodes[sl]                                   # [E, 8]
        disp = inputs[:, en]                                  # [B, E, 8, 2]
        dispc = np.ascontiguousarray(disp.transpose(1, 2, 0, 3))  # [E, 8, B, 2]
        dispc = dispc.reshape(E_CORE, 64)
        dpad = np.zeros((E_PAD, 64), np.float32)
        dpad[:E_CORE] = dispc
        spad = np.zeros((E_PAD, 144), np.float32)
        spad[:E_CORE] = shpdx[sl].reshape(E_CORE, 144)
        in_maps.append({"shp": spad, "disp": dpad})

    core_ids = list(range(N_CORES))
    res = run_bass_kernel_spmd(nc, in_maps, core_ids, trace=_want_trace)

    outs = []
    for c in range(N_CORES):
        o = res.results[c]["out"]                             # [B, E_PAD*9, 3]
        outs.append(o[:, :E_CORE * 9, :])
    full = np.concatenate(outs, axis=1)                       # [B, N_ELEM*9, 3]
    if _want_trace:
        return full, res
    return full


# revision 4
# speedup vs baseline: 80.2203x; 80.2203x over previous
"""Trainium2 kernel for the FEM kinematic (strain) layer.

Reference computation:
    disp = inputs[:, elem_nodes]                      # [B, E, 8, 2]
    dd   = einsum('egkl,bekn->begnl', shpdx, disp)    # [B, E, 9, 2, 2]
    out  = stack([dd[...,0,0], dd[...,1,1],
                  0.5*(dd[...,0,1] + dd[...,1,0])])   # [B, E*9, 3]

Sharding: elements split across 8 NeuronCores.  The host resolves the
element->node indirection (index marshalling) and ships each core an
element-major displacement block; the device streams shpdx + disp and
computes the strain products with DVE, using the identity
    S1*u + S0*v = (S0+S1)*(u+v) - S0*u - S1*v
so only 3 elementwise products are needed per (element, gauss point).
"""

import sys
import numpy as np

sys.path.insert(0, "/opt/trn_rl_repo")

import concourse.bass as bass
import concourse.bacc as bacc
import concourse.mybir as mybir
import concourse.tile as tile
from concourse.bass_utils import run_bass_kernel_spmd

B = 4
N_NODES = 1_000_000
N_ELEM = 500_000
N_GP = 9
N_EN = 8
N_CORES = 8

E_CORE = N_ELEM // N_CORES            # 62500 elements per core
P = 128                               # SBUF partitions
C = 16                                # elements per partition per chunk
CHUNK = P * C                         # 2048 elements per chunk
N_CHUNKS = -(-E_CORE // CHUNK)        # 31
E_PAD = N_CHUNKS * CHUNK              # 63488 (988 pad elements)

_compiled = None


def _build_program():
    nc = bacc.Bacc("TRN2", target_bir_lowering=False, debug=False)
    f32 = mybir.dt.float32

    # [E_PAD, 144] f32: per element (g, k, l) row-major
    s_d = nc.dram_tensor("shp", [E_PAD, 144], f32, kind="ExternalInput").ap()
    # [E_PAD, 64] f32: per element (k, b, n) row-major
    d_d = nc.dram_tensor("disp", [E_PAD, 64], f32, kind="ExternalInput").ap()
    # [B, E_PAD*9, 3] f32
    o_d = nc.dram_tensor("out", [B, E_PAD * 9, 3], f32, kind="ExternalOutput").ap()

    s_v = s_d.rearrange("(n p c) f -> n p (c f)", p=P, c=C)
    d_v = d_d.rearrange("(n p c) f -> n p (c f)", p=P, c=C)
    # out view per (b, chunk): [P, C*27]
    o_v = o_d.rearrange("b (n p x) three -> b n p (x three)", p=P, x=C * 9)

    with tile.TileContext(nc) as tc:
        with (
            tc.tile_pool(name="io", bufs=4) as io_pool,
            tc.tile_pool(name="tmp", bufs=3) as tmp_pool,
        ):
            for i in range(N_CHUNKS):
                S = io_pool.tile([P, C * 144], f32, tag="S")
                D = io_pool.tile([P, C * 64], f32, tag="D")
                nc.sync.dma_start(out=S[:], in_=s_v[i])
                nc.sync.dma_start(out=D[:], in_=d_v[i])

                Sr = S[:].rearrange("p (c g k l) -> p c g k l", c=C, g=9, k=8, l=2)
                Dr = D[:].rearrange("p (c k b n) -> p c k b n", c=C, k=8, b=B, n=2)

                # A = S0 + S1, contiguous [p, (c g k)]
                A = tmp_pool.tile([P, C * 72], f32, tag="A")
                Av = A[:].rearrange("p (c g k) -> p c g k", c=C, g=9)
                nc.vector.tensor_tensor(
                    out=Av, in0=Sr[:, :, :, :, 0], in1=Sr[:, :, :, :, 1],
                    op=mybir.AluOpType.add,
                )

                O = io_pool.tile([P, B * C * 27], f32, tag="O")
                Ov = O[:].rearrange("p (b c g t) -> p b c g t", b=B, c=C, g=9, t=3)

                for b in range(B):
                    u = Dr[:, :, :, b, 0]          # [p, C, 8]
                    v = Dr[:, :, :, b, 1]

                    W = tmp_pool.tile([P, C * 8], f32, tag="W")
                    Wv = W[:].rearrange("p (c k) -> p c k", c=C)
                    nc.gpsimd.tensor_tensor(
                        out=Wv, in0=u, in1=v, op=mybir.AluOpType.add
                    )

                    # broadcast displacement over g: [p, C, 1, 8] -> [p, C, 9, 8]
                    ub = u[:, :, None, :].to_broadcast([P, C, 9, 8])
                    vb = v[:, :, None, :].to_broadcast([P, C, 9, 8])
                    wb = Wv[:, :, None, :].to_broadcast([P, C, 9, 8])

                    T0 = tmp_pool.tile([P, C * 72], f32, tag="T")
                    T1 = tmp_pool.tile([P, C * 72], f32, tag="T")
                    T2 = tmp_pool.tile([P, C * 72], f32, tag="T")
                    T0v = T0[:].rearrange("p (c g k) -> p c g k", c=C, g=9)
                    T1v = T1[:].rearrange("p (c g k) -> p c g k", c=C, g=9)
                    T2v = T2[:].rearrange("p (c g k) -> p c g k", c=C, g=9)

                    nc.vector.tensor_tensor(
                        out=T0v, in0=Sr[:, :, :, :, 0], in1=ub,
                        op=mybir.AluOpType.mult,
                    )
                    nc.vector.tensor_tensor(
                        out=T1v, in0=Sr[:, :, :, :, 1], in1=vb,
                        op=mybir.AluOpType.mult,
                    )
                    nc.vector.tensor_tensor(
                        out=T2v, in0=Av, in1=wb, op=mybir.AluOpType.mult,
                    )

                    # xx / yy land directly in the (strided) output staging
                    nc.vector.reduce_sum(
                        out=Ov[:, b, :, :, 0], in_=T0v, axis=mybir.AxisListType.X
                    )
                    nc.vector.reduce_sum(
                        out=Ov[:, b, :, :, 1], in_=T1v, axis=mybir.AxisListType.X
                    )

                    R = tmp_pool.tile([P, C * 9], f32, tag="R")
                    Rv = R[:].rearrange("p (c g) -> p c g", c=C)
                    nc.vector.reduce_sum(out=Rv, in_=T2v, axis=mybir.AxisListType.X)
                    nc.gpsimd.tensor_tensor(
                        out=Rv, in0=Rv, in1=Ov[:, b, :, :, 0],
                        op=mybir.AluOpType.subtract,
                    )
                    nc.gpsimd.tensor_tensor(
                        out=Rv, in0=Rv, in1=Ov[:, b, :, :, 1],
                        op=mybir.AluOpType.subtract,
                    )
                    nc.scalar.activation(
                        out=Ov[:, b, :, :, 2], in_=Rv,
                        func=mybir.ActivationFunctionType.Copy, scale=0.5,
                    )

                for b in range(B):
                    nc.sync.dma_start(
                        out=o_v[b, i],
                        in_=O[:, b * C * 27:(b + 1) * C * 27],
                    )

    nc.compile()
    return nc


def _get_program():
    global _compiled
    if _compiled is None:
        _compiled = _build_program()
    return _compiled


def kernel(inputs, shpdx, elem_nodes, _want_trace=False):
    nc = _get_program()

    # Host-side index marshalling: resolve element->node indirection and
    # build per-core element-major blocks.
    in_maps = []
    for c in range(N_CORES):
        sl = slice(c * E_CORE, (c + 1) * E_CORE)
        en = elem_nodes[sl]                                   # [E, 8]
        disp = inputs[:, en]                                  # [B, E, 8, 2]
        dispc = np.ascontiguousarray(disp.transpose(1, 2, 0, 3))  # [E, 8, B, 2]
        dispc = dispc.reshape(E_CORE, 64)
        dpad = np.zeros((E_PAD, 64), np.float32)
        dpad[:E_CORE] = dispc
        spad = np.zeros((E_PAD, 144), np.float32)
        spad[:E_CORE] = shpdx[sl].reshape(E_CORE, 144)
        in_maps.append({"shp": spad, "disp": dpad})

    core_ids = list(range(N_CORES))
    res = run_bass_kernel_spmd(nc, in_maps, core_ids, trace=_want_trace)

    outs = []
    for c in range(N_CORES):
        o = res.results[c]["out"]                             # [B, E_PAD*9, 3]
        outs.append(o[:, :E_CORE * 9, :])
    full = np.concatenate(outs, axis=1)                       # [B, N_ELEM*9, 3]
    if _want_trace:
        return full, res
    return full
